# revision 23
# baseline (speedup 1.0000x reference)
"""Trainium2 Bass kernel for nn_GATLayer_58291296141986.

Math: the reference computes
    xt = (x @ W.T).reshape(B, N, H, D)            # B=32, N=10, H=8, D=8
    out[b,n,h,m] = relu(sum_k xt[b,n,h,k] * adj[b,n,m])
adj does not depend on k, so sum_k xt[b,n,h,k] = x[b,n,:] @ Wsum[h,:]
with Wsum[h] = sum_d W[h*8+d].  The whole problem collapses to
    s = x2 @ Wsum.T        # (320, 65536) @ (65536, 8)
    out[t, h*10+m] = relu(s[t,h] * adj[t,m])
which is memory-bound on reading x (84MB) + W (17MB).

Sharding: tensor-parallel over in_dim (k).  Each of the 8 cores reads a
disjoint 8192-wide k-slice of x (10.5MB) and W (2MB) and computes a
partial s^T (8, 320) -- every input byte is read exactly once across the
chip (~12.6MB/core, the memory roofline).  The cross-core reduction of
the 10KB partials is done in a second, tiny SPMD launch: the host hands
core h the 8 partial rows of head h (pure data movement), and the device
folds them with a ones-matmul (which also replicates the summed row onto
10 PSUM partitions), multiplies by adj^T and applies relu.  Core h thus
produces the 10 output columns of head h for all 320 tokens and the host
concatenates the 8 head slices.  (A single-launch variant with an
on-device AllToAll was measured ~30us slower: the collectives firmware's
entry barrier alone costs ~60us on this runtime.)

Device layout trick: the PE contracts over the partition axis, but x in
DRAM is token-major.  The host pre-swizzles each core's x slice to
    xs[p, j*320 + t] = x2[t, c*8192 + p*64 + j]   (p in 0..128, j in 0..64)
so one matmul per j (lhsT = Wsum slice (128,8), rhs = xs slice (128,320))
accumulates s^T over 64 PSUM-accumulated matmuls, with the xs DMA
arriving in 8 j-major chunks that pipeline against the PE.  W is likewise
pre-swizzled so an on-device reduce over the innermost 8 (the head's D
entries) yields Wsum in exactly the lhsT layout needed.  Matmul operands
are float32r: same fp32 bytes, single-pass PE matmul at 1 cycle/row
(plain fp32 is 4 cycles/row), costing ~1e-4 relative error.
"""

import numpy as np

import concourse.bass as bass
import concourse.mybir as mybir
import concourse.tile as tile
from concourse import bacc
from concourse.bass_utils import run_bass_kernel_spmd

B, NN, IN_DIM, OUT_DIM, HEADS = 32, 10, 65536, 64, 8
NCORES = 8
T = B * NN                 # 320 tokens
KS = IN_DIM // NCORES      # 8192 contraction slice per core
JW = KS // 128             # 64 j-steps per core
NCHUNK = 16                # xs DMA chunks
JC = JW // NCHUNK          # j-steps per chunk
F32 = mybir.dt.float32
F32R = mybir.dt.float32r


def build_main():
    """Launch 1: per-core partial s^T = (x k-slice) @ (Wsum k-slice)^T."""
    nc = bacc.Bacc("TRN2", debug=False, num_devices=NCORES, target_bir_lowering=False)

    xs_d = nc.dram_tensor("xs", [128, JW * T], F32R, kind="ExternalInput").ap()
    ws_d = nc.dram_tensor("ws", [128, JW * HEADS * 8], F32, kind="ExternalInput").ap()
    part_d = nc.dram_tensor("part", [HEADS, T], F32, kind="ExternalOutput").ap()

    with tile.TileContext(nc) as tc:
        with (
            tc.tile_pool(name="xp", bufs=NCHUNK) as xp,
            tc.tile_pool(name="wp", bufs=1) as wp,
            tc.tile_pool(name="aux", bufs=1) as aux,
            tc.tile_pool(name="pp", bufs=1, space="PSUM") as pp,
        ):
            # W in 4 pieces (DMA + head-sum reduce each) so the first matmuls
            # only wait for the first piece, not the whole 2MB of W
            NWP = 4
            JP = JW // NWP
            wsums = []
            for i in range(NWP):
                wst = wp.tile(
                    [128, JP * HEADS * 8], F32, name=f"wst{i}", tag="wst", bufs=NWP
                )
                nc.scalar.dma_start(
                    wst[:], ws_d[:, i * JP * HEADS * 8 : (i + 1) * JP * HEADS * 8]
                )
                wsum = wp.tile(
                    [128, JP * HEADS], F32R, name=f"wsum{i}", tag="wsum", bufs=NWP
                )
                with nc.allow_low_precision(
                    reason="f32r rounding of Wsum is the intended matmul precision"
                ):
                    nc.vector.reduce_sum(
                        out=wsum[:].unsqueeze(2),
                        in_=wst[:].rearrange("p (a d) -> p a d", d=8),
                        axis=mybir.AxisListType.X,
                    )
                wsums.append(wsum)

            # xs chunks alternate between the two HWDGE rings (SP and ACT)
            # so descriptor generation is not serialized on one engine
            psum_s = pp.tile([HEADS, T], F32)
            for jc in range(NCHUNK):
                xt = xp.tile([128, JC * T], F32R, name=f"xt{jc}", tag="xt")
                eng = nc.sync if jc % 2 == 0 else nc.scalar
                eng.dma_start(
                    xt[:],
                    xs_d[:, jc * JC * T : (jc + 1) * JC * T],
                )
                for a in range(JC):
                    j = jc * JC + a
                    nc.tensor.matmul(
                        psum_s[:],
                        wsums[j // JP][:, (j % JP) * HEADS : (j % JP + 1) * HEADS],
                        xt[:, a * T : (a + 1) * T],
                        start=(j == 0),
                        stop=(j == JW - 1),
                    )

            s_sbT = aux.tile([HEADS, T], F32)
            nc.vector.tensor_copy(s_sbT[:], psum_s[:])
            nc.sync.dma_start(part_d[:], s_sbT[:])

    nc.compile()
    return nc


def build_fold():
    """Launch 2: core h folds head h's 8 partials, scales by adj^T, relu."""
    nc = bacc.Bacc("TRN2", debug=False, num_devices=NCORES, target_bir_lowering=False)

    # one merged input: rows 0-7 = the 8 partials of this core's head,
    # rows 32-41 = adj^T, rows 64-71 cols 0-9 = ones for the fold matmul
    # (bases 0/32/64: engine APs only support those start partitions)
    fin_d = nc.dram_tensor("fin", [64 + NCORES, T], F32, kind="ExternalInput").ap()
    out_d = nc.dram_tensor("out", [NN, T], F32, kind="ExternalOutput").ap()

    with tile.TileContext(nc) as tc:
        with (
            tc.tile_pool(name="aux", bufs=1) as aux,
            tc.tile_pool(name="pp", bufs=1, space="PSUM") as pp,
        ):
            fin_sb = aux.tile([64 + NCORES, T], F32)
            nc.sync.dma_start(fin_sb[:], fin_d[:])

            # ones-matmul: sums the 8 partial rows and replicates the sum
            # onto 10 PSUM partitions in one shot
            psum10 = pp.tile([NN, T], F32)
            ones_sb = aux.tile([NCORES, NN], F32)
            nc.vector.memset(ones_sb[:], 1.0)
            nc.tensor.matmul(
                psum10[:], ones_sb[:], fin_sb[:NCORES, :], start=True, stop=True
            )
            prod = aux.tile([NN, T], F32)
            nc.vector.tensor_mul(prod[:], psum10[:], fin_sb[32 : 32 + NN, :])
            res = aux.tile([NN, T], F32)
            nc.vector.tensor_relu(res[:], prod[:])
            nc.sync.dma_start(out_d[:], res[:])

    nc.compile()
    return nc


def shard_inputs(x, adj, W):
    """Host-side sharding/layout (pure data movement, no math)."""
    x2 = np.ascontiguousarray(x, dtype=np.float32).reshape(T, IN_DIM)
    # xs[c][p, j*T + t] = x2[t, c*KS + p*JW + j]
    xv = x2.reshape(T, NCORES, 128, JW).transpose(1, 2, 3, 0)  # (c, p, j, t)
    xs_all = np.ascontiguousarray(xv).reshape(NCORES, 128, JW * T)
    # ws[c][p, (j*8+h)*8+d] = W[h*8+d, c*KS + p*JW + j]
    Wv = np.ascontiguousarray(W, dtype=np.float32).reshape(HEADS, 8, NCORES, 128, JW)
    wv = Wv.transpose(2, 3, 4, 0, 1)  # (c, p, j, h, d)
    ws_all = np.ascontiguousarray(wv).reshape(NCORES, 128, JW * HEADS * 8)
    return [{"xs": xs_all[c], "ws": ws_all[c]} for c in range(NCORES)]


_NC_MAIN = None
_NC_FOLD = None


def run(x, adj, W, trace=False, **kw):
    global _NC_MAIN, _NC_FOLD
    if _NC_MAIN is None:
        _NC_MAIN = build_main()
        _NC_FOLD = build_fold()

    res1 = run_bass_kernel_spmd(
        _NC_MAIN, shard_inputs(x, adj, W), core_ids=list(range(NCORES)),
        trace=trace, **kw
    )
    # host gather/scatter of the 10KB partials: core h gets row h of every
    # core's partial s^T (pure data movement)
    parts = np.stack([res1.results[c]["part"] for c in range(NCORES)])  # (c, h, t)
    adjt = np.asarray(adj, dtype=np.float32).reshape(T, NN).T
    in_maps2 = []
    for h in range(HEADS):
        fin = np.zeros((64 + NCORES, T), dtype=np.float32)
        fin[:NCORES] = parts[:, h, :]
        fin[32 : 32 + NN] = adjt
        fin[64 : 64 + NCORES, :NN] = 1.0
        in_maps2.append({"fin": fin})
    res2 = run_bass_kernel_spmd(
        _NC_FOLD, in_maps2, core_ids=list(range(NCORES)), trace=trace, **kw
    )

    full = np.empty((T, HEADS * NN), dtype=np.float32)
    for h in range(HEADS):
        full[:, h * NN : (h + 1) * NN] = res2.results[h]["out"].T
    return full.reshape(B, NN, HEADS * NN), (res1, res2)


def kernel(x, adj, W):
    out, _ = run(x, adj, W)
    return out


# revision 24
# speedup vs baseline: 1.1102x; 1.1102x over previous
"""Trainium2 Bass kernel for nn_GATLayer_58291296141986.

Math: the reference computes
    xt = (x @ W.T).reshape(B, N, H, D)            # B=32, N=10, H=8, D=8
    out[b,n,h,m] = relu(sum_k xt[b,n,h,k] * adj[b,n,m])
adj does not depend on k, so sum_k xt[b,n,h,k] = x[b,n,:] @ Wsum[h,:]
with Wsum[h] = sum_d W[h*8+d].  The whole problem collapses to
    s = x2 @ Wsum.T        # (320, 65536) @ (65536, 8)
    out[t, h*10+m] = relu(s[t,h] * adj[t,m])
which is memory-bound on reading x (84MB) + W (17MB).

Sharding: tensor-parallel over in_dim (k).  Each of the 8 cores reads a
disjoint 8192-wide k-slice of x (10.5MB) and W (2MB) and computes a
partial s^T (8, 320) -- every input byte is read exactly once across the
chip (~12.6MB/core, the memory roofline).  The cross-core reduction of
the 10KB partials is done in a second, tiny SPMD launch: the host hands
core h the 8 partial rows of head h (pure data movement), and the device
folds them with a ones-matmul (which also replicates the summed row onto
10 PSUM partitions), multiplies by adj^T and applies relu.  Core h thus
produces the 10 output columns of head h for all 320 tokens and the host
concatenates the 8 head slices.  (A single-launch variant with an
on-device AllToAll was measured ~30us slower: the collectives firmware's
entry barrier alone costs ~60us on this runtime.)

Device layout trick: the PE contracts over the partition axis, but x in
DRAM is token-major.  The host pre-swizzles each core's x slice to
    xs[p, j*320 + t] = x2[t, c*8192 + p*64 + j]   (p in 0..128, j in 0..64)
so one matmul per j (lhsT = Wsum slice (128,8), rhs = xs slice (128,320))
accumulates s^T over 64 PSUM-accumulated matmuls, with the xs DMA
arriving in 8 j-major chunks that pipeline against the PE.  W is likewise
pre-swizzled so an on-device reduce over the innermost 8 (the head's D
entries) yields Wsum in exactly the lhsT layout needed.  Matmul operands
are float32r: same fp32 bytes, single-pass PE matmul at 1 cycle/row
(plain fp32 is 4 cycles/row), costing ~1e-4 relative error.
"""

import numpy as np

import concourse.bass as bass
import concourse.mybir as mybir
import concourse.tile as tile
from concourse import bacc
from concourse.bass_utils import run_bass_kernel_spmd

B, NN, IN_DIM, OUT_DIM, HEADS = 32, 10, 65536, 64, 8
NCORES = 8
T = B * NN                 # 320 tokens
KS = IN_DIM // NCORES      # 8192 contraction slice per core
JW = KS // 128             # 64 j-steps per core
NCHUNK = 8                 # xs DMA chunks
JC = JW // NCHUNK          # j-steps per chunk
F32 = mybir.dt.float32
F32R = mybir.dt.float32r


def build_main():
    """Launch 1: per-core partial s^T = (x k-slice) @ (Wsum k-slice)^T."""
    nc = bacc.Bacc("TRN2", debug=False, num_devices=NCORES, target_bir_lowering=False)

    xs_d = nc.dram_tensor("xs", [128, JW * T], F32R, kind="ExternalInput").ap()
    ws_d = nc.dram_tensor("ws", [128, JW * HEADS * 8], F32, kind="ExternalInput").ap()
    part_d = nc.dram_tensor("part", [HEADS, T], F32, kind="ExternalOutput").ap()

    with tile.TileContext(nc) as tc:
        with (
            tc.tile_pool(name="xp", bufs=NCHUNK) as xp,
            tc.tile_pool(name="wp", bufs=1) as wp,
            tc.tile_pool(name="aux", bufs=1) as aux,
            tc.tile_pool(name="pp", bufs=1, space="PSUM") as pp,
        ):
            # W as one DMA + head-sum reduce; Wsum only gates the matmuls,
            # which are chunk-paced by the xs DMAs anyway
            NWP = 1
            JP = JW // NWP
            wsums = []
            for i in range(NWP):
                wst = wp.tile(
                    [128, JP * HEADS * 8], F32, name=f"wst{i}", tag="wst", bufs=NWP
                )
                nc.scalar.dma_start(
                    wst[:], ws_d[:, i * JP * HEADS * 8 : (i + 1) * JP * HEADS * 8]
                )
                wsum = wp.tile(
                    [128, JP * HEADS], F32R, name=f"wsum{i}", tag="wsum", bufs=NWP
                )
                with nc.allow_low_precision(
                    reason="f32r rounding of Wsum is the intended matmul precision"
                ):
                    nc.vector.reduce_sum(
                        out=wsum[:].unsqueeze(2),
                        in_=wst[:].rearrange("p (a d) -> p a d", d=8),
                        axis=mybir.AxisListType.X,
                    )
                wsums.append(wsum)

            # xs chunks alternate between the two HWDGE rings (SP and ACT)
            # so descriptor generation is not serialized on one engine
            psum_s = pp.tile([HEADS, T], F32)
            for jc in range(NCHUNK):
                xt = xp.tile([128, JC * T], F32R, name=f"xt{jc}", tag="xt")
                eng = nc.sync if jc % 2 == 0 else nc.scalar
                eng.dma_start(
                    xt[:],
                    xs_d[:, jc * JC * T : (jc + 1) * JC * T],
                )
                for a in range(JC):
                    j = jc * JC + a
                    nc.tensor.matmul(
                        psum_s[:],
                        wsums[j // JP][:, (j % JP) * HEADS : (j % JP + 1) * HEADS],
                        xt[:, a * T : (a + 1) * T],
                        start=(j == 0),
                        stop=(j == JW - 1),
                    )

            s_sbT = aux.tile([HEADS, T], F32)
            nc.vector.tensor_copy(s_sbT[:], psum_s[:])
            nc.sync.dma_start(part_d[:], s_sbT[:])

    nc.compile()
    return nc


def build_fold():
    """Launch 2: core h folds head h's 8 partials, scales by adj^T, relu."""
    nc = bacc.Bacc("TRN2", debug=False, num_devices=NCORES, target_bir_lowering=False)

    # one merged input: rows 0-7 = the 8 partials of this core's head,
    # rows 32-41 = adj^T, rows 64-71 cols 0-9 = ones for the fold matmul
    # (bases 0/32/64: engine APs only support those start partitions)
    fin_d = nc.dram_tensor("fin", [64 + NCORES, T], F32, kind="ExternalInput").ap()
    out_d = nc.dram_tensor("out", [NN, T], F32, kind="ExternalOutput").ap()

    with tile.TileContext(nc) as tc:
        with (
            tc.tile_pool(name="aux", bufs=1) as aux,
            tc.tile_pool(name="pp", bufs=1, space="PSUM") as pp,
        ):
            fin_sb = aux.tile([64 + NCORES, T], F32)
            nc.sync.dma_start(fin_sb[:], fin_d[:])

            # ones-matmul: sums the 8 partial rows and replicates the sum
            # onto 10 PSUM partitions in one shot
            psum10 = pp.tile([NN, T], F32)
            ones_sb = aux.tile([NCORES, NN], F32)
            nc.vector.memset(ones_sb[:], 1.0)
            nc.tensor.matmul(
                psum10[:], ones_sb[:], fin_sb[:NCORES, :], start=True, stop=True
            )
            prod = aux.tile([NN, T], F32)
            nc.vector.tensor_mul(prod[:], psum10[:], fin_sb[32 : 32 + NN, :])
            res = aux.tile([NN, T], F32)
            nc.vector.tensor_relu(res[:], prod[:])
            nc.sync.dma_start(out_d[:], res[:])

    nc.compile()
    return nc


def shard_inputs(x, adj, W):
    """Host-side sharding/layout (pure data movement, no math)."""
    x2 = np.ascontiguousarray(x, dtype=np.float32).reshape(T, IN_DIM)
    # xs[c][p, j*T + t] = x2[t, c*KS + p*JW + j]
    xv = x2.reshape(T, NCORES, 128, JW).transpose(1, 2, 3, 0)  # (c, p, j, t)
    xs_all = np.ascontiguousarray(xv).reshape(NCORES, 128, JW * T)
    # ws[c][p, (j*8+h)*8+d] = W[h*8+d, c*KS + p*JW + j]
    Wv = np.ascontiguousarray(W, dtype=np.float32).reshape(HEADS, 8, NCORES, 128, JW)
    wv = Wv.transpose(2, 3, 4, 0, 1)  # (c, p, j, h, d)
    ws_all = np.ascontiguousarray(wv).reshape(NCORES, 128, JW * HEADS * 8)
    return [{"xs": xs_all[c], "ws": ws_all[c]} for c in range(NCORES)]


_NC_MAIN = None
_NC_FOLD = None


def run(x, adj, W, trace=False, **kw):
    global _NC_MAIN, _NC_FOLD
    if _NC_MAIN is None:
        _NC_MAIN = build_main()
        _NC_FOLD = build_fold()

    res1 = run_bass_kernel_spmd(
        _NC_MAIN, shard_inputs(x, adj, W), core_ids=list(range(NCORES)),
        trace=trace, **kw
    )
    # host gather/scatter of the 10KB partials: core h gets row h of every
    # core's partial s^T (pure data movement)
    parts = np.stack([res1.results[c]["part"] for c in range(NCORES)])  # (c, h, t)
    adjt = np.asarray(adj, dtype=np.float32).reshape(T, NN).T
    in_maps2 = []
    for h in range(HEADS):
        fin = np.zeros((64 + NCORES, T), dtype=np.float32)
        fin[:NCORES] = parts[:, h, :]
        fin[32 : 32 + NN] = adjt
        fin[64 : 64 + NCORES, :NN] = 1.0
        in_maps2.append({"fin": fin})
    res2 = run_bass_kernel_spmd(
        _NC_FOLD, in_maps2, core_ids=list(range(NCORES)), trace=trace, **kw
    )

    full = np.empty((T, HEADS * NN), dtype=np.float32)
    for h in range(HEADS):
        full[:, h * NN : (h + 1) * NN] = res2.results[h]["out"].T
    return full.reshape(B, NN, HEADS * NN), (res1, res2)


def kernel(x, adj, W):
    out, _ = run(x, adj, W)
    return out


# revision 25
# speedup vs baseline: 1.2302x; 1.1081x over previous
"""Trainium2 Bass kernel for nn_GATLayer_58291296141986.

Math: the reference computes
    xt = (x @ W.T).reshape(B, N, H, D)            # B=32, N=10, H=8, D=8
    out[b,n,h,m] = relu(sum_k xt[b,n,h,k] * adj[b,n,m])
adj does not depend on k, so sum_k xt[b,n,h,k] = x[b,n,:] @ Wsum[h,:]
with Wsum[h] = sum_d W[h*8+d].  The whole problem collapses to
    s = x2 @ Wsum.T        # (320, 65536) @ (65536, 8)
    out[t, h*10+m] = relu(s[t,h] * adj[t,m])
which is memory-bound on reading x (84MB) + W (17MB).

Sharding: tensor-parallel over in_dim (k).  Each of the 8 cores reads a
disjoint 8192-wide k-slice of x (10.5MB) and W (2MB) and computes a
partial s^T (8, 320) -- every input byte is read exactly once across the
chip (~12.6MB/core, the memory roofline).  The cross-core reduction of
the 10KB partials is done in a second, tiny SPMD launch: the host hands
core h the 8 partial rows of head h (pure data movement), and the device
folds them with a ones-matmul (which also replicates the summed row onto
10 PSUM partitions), multiplies by adj^T and applies relu.  Core h thus
produces the 10 output columns of head h for all 320 tokens and the host
concatenates the 8 head slices.  (A single-launch variant with an
on-device AllToAll was measured ~30us slower: the collectives firmware's
entry barrier alone costs ~60us on this runtime.)

Device layout trick: the PE contracts over the partition axis, but x in
DRAM is token-major.  The host pre-swizzles each core's x slice to
    xs[p, j*320 + t] = x2[t, c*8192 + p*64 + j]   (p in 0..128, j in 0..64)
so one matmul per j (lhsT = Wsum slice (128,8), rhs = xs slice (128,320))
accumulates s^T over 64 PSUM-accumulated matmuls, with the xs DMA
arriving in 8 j-major chunks that pipeline against the PE.  W is likewise
pre-swizzled so an on-device reduce over the innermost 8 (the head's D
entries) yields Wsum in exactly the lhsT layout needed.  Matmul operands
are float32r: same fp32 bytes, single-pass PE matmul at 1 cycle/row
(plain fp32 is 4 cycles/row), costing ~1e-4 relative error.
"""

import numpy as np

import concourse.bass as bass
import concourse.mybir as mybir
import concourse.tile as tile
from concourse import bacc
from concourse.bass_utils import run_bass_kernel_spmd

B, NN, IN_DIM, OUT_DIM, HEADS = 32, 10, 65536, 64, 8
NCORES = 8
T = B * NN                 # 320 tokens
KS = IN_DIM // NCORES      # 8192 contraction slice per core
JW = KS // 128             # 64 j-steps per core
NCHUNK = 8                 # xs DMA chunks
JC = JW // NCHUNK          # j-steps per chunk
F32 = mybir.dt.float32
F32R = mybir.dt.float32r


def build_main():
    """Launch 1: per-core partial s^T = (x k-slice) @ (Wsum k-slice)^T."""
    nc = bacc.Bacc("TRN2", debug=False, num_devices=NCORES, target_bir_lowering=False)

    xs_d = nc.dram_tensor("xs", [128, JW * T], F32R, kind="ExternalInput").ap()
    ws_d = nc.dram_tensor("ws", [128, JW * HEADS * 8], F32, kind="ExternalInput").ap()
    part_d = nc.dram_tensor("part", [HEADS, T], F32, kind="ExternalOutput").ap()

    with tile.TileContext(nc) as tc:
        with (
            tc.tile_pool(name="xp", bufs=NCHUNK) as xp,
            tc.tile_pool(name="wp", bufs=1) as wp,
            tc.tile_pool(name="aux", bufs=1) as aux,
            tc.tile_pool(name="pp", bufs=1, space="PSUM") as pp,
        ):
            # W as one DMA + head-sum reduce; Wsum only gates the matmuls,
            # which are chunk-paced by the xs DMAs anyway
            NWP = 1
            JP = JW // NWP
            wsums = []
            for i in range(NWP):
                wst = wp.tile(
                    [128, JP * HEADS * 8], F32, name=f"wst{i}", tag="wst", bufs=NWP
                )
                nc.scalar.dma_start(
                    wst[:], ws_d[:, i * JP * HEADS * 8 : (i + 1) * JP * HEADS * 8]
                )
                wsum = wp.tile(
                    [128, JP * HEADS], F32R, name=f"wsum{i}", tag="wsum", bufs=NWP
                )
                with nc.allow_low_precision(
                    reason="f32r rounding of Wsum is the intended matmul precision"
                ):
                    nc.vector.reduce_sum(
                        out=wsum[:].unsqueeze(2),
                        in_=wst[:].rearrange("p (a d) -> p a d", d=8),
                        axis=mybir.AxisListType.X,
                    )
                wsums.append(wsum)

            # xs chunks alternate between the two HWDGE rings (SP and ACT)
            # so descriptor generation is not serialized on one engine
            psum_s = pp.tile([HEADS, T], F32)
            for jc in range(NCHUNK):
                xt = xp.tile([128, JC * T], F32R, name=f"xt{jc}", tag="xt")
                eng = nc.sync if jc % 2 == 0 else nc.scalar
                eng.dma_start(
                    xt[:],
                    xs_d[:, jc * JC * T : (jc + 1) * JC * T],
                )
                for a in range(JC):
                    j = jc * JC + a
                    nc.tensor.matmul(
                        psum_s[:],
                        wsums[j // JP][:, (j % JP) * HEADS : (j % JP + 1) * HEADS],
                        xt[:, a * T : (a + 1) * T],
                        start=(j == 0),
                        stop=(j == JW - 1),
                    )

            s_sbT = aux.tile([HEADS, T], F32)
            nc.vector.tensor_copy(s_sbT[:], psum_s[:])
            nc.sync.dma_start(part_d[:], s_sbT[:])

    nc.compile()
    return nc


def build_fold():
    """Launch 2: core h folds head h's 8 partials, scales by adj^T, relu."""
    nc = bacc.Bacc("TRN2", debug=False, num_devices=NCORES, target_bir_lowering=False)

    # merged input: rows 0-7 = the 8 partials of this core's head,
    # rows 32-41 = adj^T (bases 0/32: engine APs only support 0/32/64)
    fin_d = nc.dram_tensor("fin", [32 + NN, T], F32R, kind="ExternalInput").ap()
    ones_d = nc.dram_tensor("onesr", [NCORES, NN], F32R, kind="ExternalInput").ap()
    out_d = nc.dram_tensor("out", [NN, T], F32, kind="ExternalOutput").ap()

    with tile.TileContext(nc) as tc:
        with (
            tc.tile_pool(name="aux", bufs=1) as aux,
            tc.tile_pool(name="pp", bufs=1, space="PSUM") as pp,
        ):
            fin_sb = aux.tile([32 + NN, T], F32R)
            nc.sync.dma_start(fin_sb[:], fin_d[:])
            ones_sb = aux.tile([NCORES, NN], F32R)
            nc.scalar.dma_start(ones_sb[:], ones_d[:])

            # ones-matmul: sums the 8 partial rows and replicates the sum
            # onto 10 PSUM partitions in one shot
            psum10 = pp.tile([NN, T], F32)
            nc.tensor.matmul(
                psum10[:], ones_sb[:], fin_sb[:NCORES, :], start=True, stop=True
            )
            prod = aux.tile([NN, T], F32)
            nc.vector.tensor_mul(
                prod[:], psum10[:], fin_sb[32 : 32 + NN, :].bitcast(F32)
            )
            res = aux.tile([NN, T], F32)
            nc.vector.tensor_relu(res[:], prod[:])
            nc.sync.dma_start(out_d[:], res[:])

    nc.compile()
    return nc


def shard_inputs(x, adj, W):
    """Host-side sharding/layout (pure data movement, no math)."""
    x2 = np.ascontiguousarray(x, dtype=np.float32).reshape(T, IN_DIM)
    # xs[c][p, j*T + t] = x2[t, c*KS + p*JW + j]
    xv = x2.reshape(T, NCORES, 128, JW).transpose(1, 2, 3, 0)  # (c, p, j, t)
    xs_all = np.ascontiguousarray(xv).reshape(NCORES, 128, JW * T)
    # ws[c][p, (j*8+h)*8+d] = W[h*8+d, c*KS + p*JW + j]
    Wv = np.ascontiguousarray(W, dtype=np.float32).reshape(HEADS, 8, NCORES, 128, JW)
    wv = Wv.transpose(2, 3, 4, 0, 1)  # (c, p, j, h, d)
    ws_all = np.ascontiguousarray(wv).reshape(NCORES, 128, JW * HEADS * 8)
    return [{"xs": xs_all[c], "ws": ws_all[c]} for c in range(NCORES)]


_NC_MAIN = None
_NC_FOLD = None


def run(x, adj, W, trace=False, **kw):
    global _NC_MAIN, _NC_FOLD
    if _NC_MAIN is None:
        _NC_MAIN = build_main()
        _NC_FOLD = build_fold()

    res1 = run_bass_kernel_spmd(
        _NC_MAIN, shard_inputs(x, adj, W), core_ids=list(range(NCORES)),
        trace=trace, **kw
    )
    # host gather/scatter of the 10KB partials: core h gets row h of every
    # core's partial s^T (pure data movement)
    parts = np.stack([res1.results[c]["part"] for c in range(NCORES)])  # (c, h, t)
    adjt = np.asarray(adj, dtype=np.float32).reshape(T, NN).T
    ones = np.ones((NCORES, NN), dtype=np.float32)
    in_maps2 = []
    for h in range(HEADS):
        fin = np.zeros((32 + NN, T), dtype=np.float32)
        fin[:NCORES] = parts[:, h, :]
        fin[32:] = adjt
        in_maps2.append({"fin": fin, "onesr": ones})
    res2 = run_bass_kernel_spmd(
        _NC_FOLD, in_maps2, core_ids=list(range(NCORES)), trace=trace, **kw
    )

    full = np.empty((T, HEADS * NN), dtype=np.float32)
    for h in range(HEADS):
        full[:, h * NN : (h + 1) * NN] = res2.results[h]["out"].T
    return full.reshape(B, NN, HEADS * NN), (res1, res2)


def kernel(x, adj, W):
    out, _ = run(x, adj, W)
    return out
